# revision 9
# baseline (speedup 1.0000x reference)
"""Causal self-attention (B=4, T=2048, C=2048, H=16, rope) on 8 TRN2 NeuronCores.

Sharding: core = (batch b, head-group g) with b in 0..3, g in 0..1. Each core
owns 8 heads of one batch: computes its qkv projection shard, rope, causal
attention, and a partial out-projection (contracting only its 1024 columns of
the 2048-dim head-concat axis). Host sums the two partials per batch
(the "all-reduce after out_proj"), and reassembles k (post-rope) and v.

Per-core dataflow (all matmuls in fp32r on the PE):
  xT [C,T] resident in SBUF.
  v [T,1024] produced in natural layout (lhsT = xT block), written to the v
    output and re-read per head during attention.
  qT,kT [1024,T] produced head-transposed (lhsT = wqkvT panel), rope applied
    at psum-evict time (partition-half swap via SBUF-SBUF DMA), spilled to
    DRAM (the kT spill doubles as the k output).
  Attention per head: S = qT_blk.T @ kT chunks -> causal mask on diag block ->
    softmax (max on DVE, exp+row-sum fused on ACT, 1/sum applied to P) ->
    PE-transpose P blocks -> oT accum = v_blk.T @ P_T, spilled per head.
  out_proj: out[t,o] accumulated over 8 heads (lhsT = oT block, rhs = w_outT).
"""

import os
import sys
import types
import contextlib

sys.path.insert(0, "/opt/trn_rl_repo")

import numpy as np
import ml_dtypes

import concourse.bacc as bacc
import concourse.mybir as mybir
import concourse.tile as tile
from concourse.bass_utils import run_bass_kernel_spmd

F32 = mybir.dt.float32
F32R = mybir.dt.float32r
BF16 = mybir.dt.bfloat16

B, T, C = 4, 2048, 2048
H, HD = 16, 128
HG = 8                      # heads per core
CB = C // 128               # 16 contraction blocks
TB = T // 128               # 16 time blocks
TS = T // 512               # 4 time supertiles
SCALE = 1.0 / np.sqrt(HD)
THETA = 10000.0

LAST_RESULTS = None


def _install_ntff_hook():
    """Make run_bass_kernel_spmd(trace=True) work: register the axon NTFF
    profile hook that the image's antenv package lacks."""
    if "antenv.axon_hooks" in sys.modules:
        return True
    try:
        sys.path.insert(0, "/root/.axon_site")
        from trn_agent_boot.trn_boot import _ntff_profile_via_ctypes
        hook = _ntff_profile_via_ctypes("/opt/axon/libaxon_pjrt.so")
    except Exception:
        return False
    if hook is None:
        return False
    mod = types.ModuleType("antenv.axon_hooks")
    mod._hook = hook
    mod.get_axon_ntff_profile_hook = lambda: mod._hook
    mod.set_axon_ntff_profile_hook = lambda h: setattr(mod, "_hook", h)
    sys.modules["antenv.axon_hooks"] = mod
    import antenv
    antenv.axon_hooks = mod
    return True


def _r(ap):
    return ap.bitcast(F32R)


def build_nc():
    nc = bacc.Bacc(None, target_bir_lowering=False)

    xT_d = nc.declare_dram_parameter("xT", [C, T], BF16, isOutput=False)
    wqkvT_d = nc.declare_dram_parameter("wqkvT", [C, 3 * HG * HD], BF16, isOutput=False)
    w_outT_d = nc.declare_dram_parameter("w_outT", [HG * HD, C], BF16, isOutput=False)
    cosT_d = nc.declare_dram_parameter("cosT", [HD, T], F32R, isOutput=False)
    sinT_d = nc.declare_dram_parameter("sinT", [HD, T], F32R, isOutput=False)
    maskg_d = nc.declare_dram_parameter("maskg", [128, 4, 512], F32, isOutput=False)

    outp_d = nc.declare_dram_parameter("outp", [T, C], F32, isOutput=True)
    kT_out_d = nc.declare_dram_parameter("kT_out", [HG * HD, T], F32R, isOutput=True)
    v_out_d = nc.declare_dram_parameter("v_out", [T, HG * HD], F32R, isOutput=True)

    q16_scr = nc.dram_tensor("q16_scr", [HG * HD, T], BF16)
    k16_scr = nc.dram_tensor("k16_scr", [HG * HD, T], BF16)
    v16_scr = nc.dram_tensor("v16_scr", [T, HG * HD], BF16)
    oT_scr = nc.dram_tensor("oT_scr", [HG * HD, T], BF16)

    with tile.TileContext(nc) as tc:
        with contextlib.ExitStack() as xstack:
            xpool = xstack.enter_context(tc.tile_pool(name="xpool", bufs=1))
            x_sb = xpool.tile([128, CB, T], BF16, tag="x")
            for ts in range(TS):
                for cb in range(CB):
                    nc.sync.dma_start(
                        out=x_sb[:, cb, ts * 512:(ts + 1) * 512],
                        in_=xT_d[cb * 128:(cb + 1) * 128, ts * 512:(ts + 1) * 512],
                    )

            # ---------------- phase V: v = x @ Wv.T (natural layout) -------
            with contextlib.ExitStack() as ph:
                wvpool = ph.enter_context(tc.tile_pool(name="wvpool", bufs=1))
                psv = ph.enter_context(tc.tile_pool(name="psv", bufs=3, space="PSUM"))
                vev = ph.enter_context(tc.tile_pool(name="vev", bufs=3))
                for vs in range(2):
                    wv_sb = wvpool.tile([128, CB, 512], BF16, tag="wv")
                    for cb in range(CB):
                        nc.sync.dma_start(
                            out=wv_sb[:, cb, :],
                            in_=wqkvT_d[cb * 128:(cb + 1) * 128,
                                        2048 + vs * 512:2048 + (vs + 1) * 512],
                        )
                    for tb in range(TB):
                        ps = psv.tile([128, 512], F32, tag="ps")
                        for cb in range(CB):
                            nc.tensor.matmul(
                                ps[:],
                                (x_sb[:, cb, tb * 128:(tb + 1) * 128]),
                                (wv_sb[:, cb, :]),
                                start=(cb == 0), stop=(cb == CB - 1),
                            )
                        vt = vev.tile([128, 512], F32R, tag="vout")
                        nc.scalar.copy(out=vt[:], in_=ps[:])
                        nc.sync.dma_start(
                            out=v_out_d[tb * 128:(tb + 1) * 128,
                                        vs * 512:(vs + 1) * 512],
                            in_=vt[:])
                        vt16 = vev.tile([128, 512], BF16, tag="vout16")
                        nc.vector.tensor_copy(out=vt16[:], in_=ps[:])
                        nc.sync.dma_start(
                            out=v16_scr[tb * 128:(tb + 1) * 128,
                                        vs * 512:(vs + 1) * 512],
                            in_=vt16[:])

            # ---------------- phase QK: qT,kT + rope ----------------------
            with contextlib.ExitStack() as ph:
                trig = ph.enter_context(tc.tile_pool(name="trig", bufs=1))
                cos_sb = trig.tile([HD, T], F32R, tag="cos")
                sin_sb = trig.tile([HD, T], F32R, tag="sin")
                nc.sync.dma_start(out=cos_sb[:], in_=cosT_d[:])
                nc.sync.dma_start(out=sin_sb[:], in_=sinT_d[:])

                wpool = ph.enter_context(tc.tile_pool(name="wpool", bufs=2))
                psqk = ph.enter_context(
                    tc.tile_pool(name="psqk", bufs=3, space="PSUM"))
                rpool = ph.enter_context(tc.tile_pool(name="rpool", bufs=3))

                for jb in range(2 * HG):
                    w_sb = wpool.tile([128, CB, 128], BF16, tag="wqk")
                    nc.sync.dma_start(
                        out=w_sb[:],
                        in_=wqkvT_d[:, jb * 128:(jb + 1) * 128].rearrange(
                            "(cb c) j -> c cb j", c=128),
                    )
                    for ts in range(TS):
                        sl = slice(ts * 512, (ts + 1) * 512)
                        ps = psqk.tile([128, 512], F32, tag="ps")
                        for cb in range(CB):
                            nc.tensor.matmul(
                                ps[:], (w_sb[:, cb, :]), (x_sb[:, cb, sl]),
                                start=(cb == 0), stop=(cb == CB - 1),
                            )
                        tmp = rpool.tile([128, 512], F32R, tag="tmp")
                        nc.scalar.copy(out=tmp[:], in_=ps[:])
                        rot = rpool.tile([128, 512], F32R, tag="rot")
                        nc.sync.dma_start(out=rot[0:64, :], in_=tmp[64:128, :])
                        nc.sync.dma_start(out=rot[64:128, :], in_=tmp[0:64, :])
                        nc.vector.tensor_mul(out=tmp[:], in0=tmp[:], in1=cos_sb[:, sl])
                        nc.vector.tensor_mul(out=rot[:], in0=rot[:], in1=sin_sb[:, sl])
                        nc.vector.tensor_add(out=tmp[:], in0=tmp[:], in1=rot[:])
                        if jb >= HG:
                            nc.sync.dma_start(
                                out=kT_out_d[(jb % HG) * 128:(jb % HG + 1) * 128, sl],
                                in_=tmp[:])
                        tmp16 = rpool.tile([128, 512], BF16, tag="tmp16")
                        nc.vector.tensor_copy(out=tmp16[:], in_=tmp[:])
                        dst16 = q16_scr if jb < HG else k16_scr
                        nc.sync.dma_start(
                            out=dst16[(jb % HG) * 128:(jb % HG + 1) * 128, sl],
                            in_=tmp16[:])

        # ---------------- phase A: attention per head ----------------------
        # Transposed-score formulation: S_T[j, i] = k_j.T @ q_i needs no
        # P transposes -- v_sb [j, d] is directly the PV lhsT and a ones-
        # vector matmul accumulates the softmax denominators over j.
        with contextlib.ExitStack() as ph:
            small = ph.enter_context(tc.tile_pool(name="small", bufs=1))
            maskg_sb = small.tile([128, 4, 512], F32, tag="maskg")
            nc.sync.dma_start(out=maskg_sb[:], in_=maskg_d[:])
            ones16 = small.tile([128, 1], BF16, tag="ones16")
            nc.vector.memset(ones16[:], 1.0)

            qkpool = ph.enter_context(tc.tile_pool(name="qkpool", bufs=2))
            vpool = ph.enter_context(tc.tile_pool(name="vpool", bufs=2))
            stpool = ph.enter_context(tc.tile_pool(name="stpool", bufs=4))
            otpool = ph.enter_context(tc.tile_pool(name="otpool", bufs=2))
            stats = ph.enter_context(tc.tile_pool(name="stats", bufs=3))
            bpool = ph.enter_context(tc.tile_pool(name="bpool", bufs=2))
            psS = ph.enter_context(tc.tile_pool(name="psS", bufs=3, space="PSUM"))
            psO = ph.enter_context(tc.tile_pool(name="psO", bufs=2, space="PSUM"))
            psSum = ph.enter_context(tc.tile_pool(name="psSum", bufs=2, space="PSUM"))

            for h in range(HG):
                hsl = slice(h * 128, (h + 1) * 128)
                q_sb = qkpool.tile([128, T], BF16, tag="q")
                nc.sync.dma_start(out=q_sb[:], in_=q16_scr[hsl, :])
                k_sb = qkpool.tile([128, T], BF16, tag="k")
                nc.sync.dma_start(out=k_sb[:], in_=k16_scr[hsl, :])
                v_sb = vpool.tile([128, TB, 128], BF16, tag="v")
                nc.sync.dma_start(
                    out=v_sb[:],
                    in_=v16_scr[:, hsl].rearrange("(tb t) d -> t tb d", t=128))
                oT_sb = otpool.tile([128, T], BF16, tag="oT")

                for ig in range(4):
                    igsl = slice(ig * 512, (ig + 1) * 512)
                    njb = 4 * ig + 4
                    po = psO.tile([128, 512], F32, tag="psO")
                    sums = psSum.tile([1, 512], F32, tag="sums")

                    def s_exp(jb):
                        # S_T block -> mask (in-group only) -> exp -> bf16 P_T
                        ps = psS.tile([128, 512], F32, tag="psS")
                        nc.tensor.matmul(
                            ps[:],
                            (k_sb[:, jb * 128:(jb + 1) * 128]),
                            (q_sb[:, igsl]),
                            start=True, stop=True,
                        )
                        p = jb - 4 * ig
                        if p >= 0:
                            w = (p + 1) * 128
                            nc.vector.tensor_add(
                                out=ps[:, :w], in0=ps[:, :w],
                                in1=maskg_sb[:, p, :w])
                        s16t = stpool.tile([128, 512], BF16, tag="s16t")
                        nc.scalar.activation(
                            out=s16t[:], in_=ps[:],
                            func=mybir.ActivationFunctionType.Exp,
                            bias=0.0, scale=SCALE)
                        return s16t

                    def sum_pv(jb, s16t):
                        nc.tensor.matmul(
                            sums[:], (ones16[:]), (s16t[:]),
                            start=(jb == 0), stop=(jb == njb - 1),
                        )
                        nc.tensor.matmul(
                            po[:], (v_sb[:, jb, :]), (s16t[:]),
                            start=(jb == 0), stop=(jb == njb - 1),
                        )

                    # software pipeline: S/exp of jb+1 ahead of sum/PV of jb
                    prev = s_exp(0)
                    for jb in range(1, njb):
                        cur = s_exp(jb)
                        sum_pv(jb - 1, prev)
                        prev = cur
                    sum_pv(njb - 1, prev)

                    rs = stats.tile([1, 512], F32, tag="rs")
                    nc.vector.reciprocal(out=rs[:], in_=sums[:])
                    rb = bpool.tile([128, 512], F32, tag="rb")
                    nc.gpsimd.partition_broadcast(rb[:], rs[:])
                    nc.vector.tensor_mul(
                        out=oT_sb[:, igsl], in0=po[:], in1=rb[:])
                nc.sync.dma_start(out=oT_scr[hsl, :], in_=oT_sb[:])

        # ---------------- phase O: out projection --------------------------
        with contextlib.ExitStack() as ph:
            wopool = ph.enter_context(tc.tile_pool(name="wopool", bufs=1))
            oipool = ph.enter_context(tc.tile_pool(name="oipool", bufs=1))
            psP = ph.enter_context(tc.tile_pool(name="psP", bufs=3, space="PSUM"))
            fpool = ph.enter_context(tc.tile_pool(name="fpool", bufs=3))

            wo_sb = wopool.tile([128, HG, C], BF16, tag="wo")
            oi_sb = oipool.tile([128, HG, T], BF16, tag="oi")
            for h in range(HG):
                nc.sync.dma_start(
                    out=wo_sb[:, h, :], in_=w_outT_d[h * 128:(h + 1) * 128, :])
                nc.sync.dma_start(
                    out=oi_sb[:, h, :], in_=oT_scr[h * 128:(h + 1) * 128, :])

            for tb in range(TB):
                for os_ in range(TS):
                    osl = slice(os_ * 512, (os_ + 1) * 512)
                    ps = psP.tile([128, 512], F32, tag="psP")
                    for h in range(HG):
                        nc.tensor.matmul(
                            ps[:],
                            (oi_sb[:, h, tb * 128:(tb + 1) * 128]),
                            (wo_sb[:, h, osl]),
                            start=(h == 0), stop=(h == HG - 1),
                        )
                    ft = fpool.tile([128, 512], F32, tag="fout")
                    nc.scalar.copy(out=ft[:], in_=ps[:])
                    nc.sync.dma_start(
                        out=outp_d[tb * 128:(tb + 1) * 128, osl], in_=ft[:])

    nc.finalize()
    return nc


_NC_CACHE = None


def _host_tables():
    inv_freq = 1.0 / (THETA ** (np.arange(0, HD, 2, dtype=np.float64) / HD))
    t_ar = np.arange(T, dtype=np.float64)
    emb = np.concatenate([np.outer(t_ar, inv_freq)] * 2, axis=-1)   # [T, 128]
    cosT = np.cos(emb).T.astype(np.float32).copy()
    sinT = np.sin(emb).T.astype(np.float32).copy()
    sinT[:64] *= -1.0
    # maskg[j, p, q*128+ii]: for an S_T block at in-group position p
    # (jb = 4*ig+p), query sub-block q of the 512-wide group: fully masked
    # for q < p, causal triangle (j > ii masked) for q == p, open for q > p.
    maskg = np.zeros((128, 4, 4, 128), np.float32)
    jj = np.arange(128)[:, None]
    ii = np.arange(128)[None, :]
    tri = np.where(jj > ii, np.float32(-1e30), np.float32(0.0))
    for p in range(4):
        for q in range(4):
            if q < p:
                maskg[:, p, q, :] = -1e30
            elif q == p:
                maskg[:, p, q, :] = tri
    maskg = maskg.reshape(128, 4, 512)
    return cosT, sinT, maskg


def kernel(x, wqkv, w_out):
    global _NC_CACHE, LAST_RESULTS
    x = np.ascontiguousarray(np.asarray(x, dtype=np.float32))
    wqkv = np.asarray(wqkv, dtype=np.float32)
    w_out = np.asarray(w_out, dtype=np.float32)

    if _NC_CACHE is None:
        _NC_CACHE = build_nc()
    nc = _NC_CACHE

    cosT, sinT, maskg = _host_tables()
    in_maps = []
    for core in range(8):
        b, g = core // 2, core % 2
        rows = slice(g * HG * HD, (g + 1) * HG * HD)
        wq = wqkv[0 * C:1 * C][rows]
        wk = wqkv[1 * C:2 * C][rows]
        wv = wqkv[2 * C:3 * C][rows]
        in_maps.append({
            "xT": np.ascontiguousarray(x[b].T).astype(ml_dtypes.bfloat16),
            "wqkvT": np.ascontiguousarray(
                np.concatenate([wq, wk, wv], axis=0).T).astype(ml_dtypes.bfloat16),
            "w_outT": np.ascontiguousarray(
                w_out[:, rows].T).astype(ml_dtypes.bfloat16),
            "cosT": cosT,
            "sinT": sinT,
            "maskg": maskg,
        })

    trace = bool(os.environ.get("KERNEL_TRACE"))
    if trace:
        _install_ntff_hook()
    res = run_bass_kernel_spmd(nc, in_maps, list(range(8)), trace=trace)
    LAST_RESULTS = res

    out = np.zeros((B, T, C), np.float32)
    k_full = np.empty((B, H, T, HD), np.float32)
    v_full = np.empty((B, H, T, HD), np.float32)
    for core in range(8):
        b, g = core // 2, core % 2
        r = res.results[core]
        out[b] += r["outp"]
        k_full[b, g * HG:(g + 1) * HG] = (
            r["kT_out"].reshape(HG, HD, T).transpose(0, 2, 1))
        v_full[b, g * HG:(g + 1) * HG] = (
            r["v_out"].reshape(T, HG, HD).transpose(1, 0, 2))
    return out, k_full, v_full


# revision 10
# speedup vs baseline: 1.0689x; 1.0689x over previous
"""Causal self-attention (B=4, T=2048, C=2048, H=16, rope) on 8 TRN2 NeuronCores.

Sharding: core = (batch b, head-group g) with b in 0..3, g in 0..1. Each core
owns 8 heads of one batch: computes its qkv projection shard, rope, causal
attention, and a partial out-projection (contracting only its 1024 columns of
the 2048-dim head-concat axis). Host sums the two partials per batch
(the "all-reduce after out_proj"), and reassembles k (post-rope) and v.

Per-core dataflow (all matmuls in fp32r on the PE):
  xT [C,T] resident in SBUF.
  v [T,1024] produced in natural layout (lhsT = xT block), written to the v
    output and re-read per head during attention.
  qT,kT [1024,T] produced head-transposed (lhsT = wqkvT panel), rope applied
    at psum-evict time (partition-half swap via SBUF-SBUF DMA), spilled to
    DRAM (the kT spill doubles as the k output).
  Attention per head: S = qT_blk.T @ kT chunks -> causal mask on diag block ->
    softmax (max on DVE, exp+row-sum fused on ACT, 1/sum applied to P) ->
    PE-transpose P blocks -> oT accum = v_blk.T @ P_T, spilled per head.
  out_proj: out[t,o] accumulated over 8 heads (lhsT = oT block, rhs = w_outT).
"""

import os
import sys
import types
import contextlib

sys.path.insert(0, "/opt/trn_rl_repo")

import numpy as np
import ml_dtypes

import concourse.bacc as bacc
import concourse.mybir as mybir
import concourse.tile as tile
from concourse.bass_utils import run_bass_kernel_spmd

F32 = mybir.dt.float32
F32R = mybir.dt.float32r
BF16 = mybir.dt.bfloat16

B, T, C = 4, 2048, 2048
H, HD = 16, 128
HG = 8                      # heads per core
CB = C // 128               # 16 contraction blocks
TB = T // 128               # 16 time blocks
TS = T // 512               # 4 time supertiles
SCALE = 1.0 / np.sqrt(HD)
THETA = 10000.0

LAST_RESULTS = None


def _install_ntff_hook():
    """Make run_bass_kernel_spmd(trace=True) work: register the axon NTFF
    profile hook that the image's antenv package lacks."""
    if "antenv.axon_hooks" in sys.modules:
        return True
    try:
        sys.path.insert(0, "/root/.axon_site")
        from trn_agent_boot.trn_boot import _ntff_profile_via_ctypes
        hook = _ntff_profile_via_ctypes("/opt/axon/libaxon_pjrt.so")
    except Exception:
        return False
    if hook is None:
        return False
    mod = types.ModuleType("antenv.axon_hooks")
    mod._hook = hook
    mod.get_axon_ntff_profile_hook = lambda: mod._hook
    mod.set_axon_ntff_profile_hook = lambda h: setattr(mod, "_hook", h)
    sys.modules["antenv.axon_hooks"] = mod
    import antenv
    antenv.axon_hooks = mod
    return True


def _r(ap):
    return ap.bitcast(F32R)


def build_nc():
    nc = bacc.Bacc(None, target_bir_lowering=False)

    xT_d = nc.declare_dram_parameter("xT", [C, T], BF16, isOutput=False)
    wqkvT_d = nc.declare_dram_parameter("wqkvT", [C, 3 * HG * HD], BF16, isOutput=False)
    w_outT_d = nc.declare_dram_parameter("w_outT", [HG * HD, C], BF16, isOutput=False)
    cosT_d = nc.declare_dram_parameter("cosT", [HD, T], F32R, isOutput=False)
    sinT_d = nc.declare_dram_parameter("sinT", [HD, T], F32R, isOutput=False)
    mask_d = nc.declare_dram_parameter("mask", [128, 128], F32, isOutput=False)

    outp_d = nc.declare_dram_parameter("outp", [T, C], F32, isOutput=True)
    kT_out_d = nc.declare_dram_parameter("kT_out", [HG * HD, T], F32R, isOutput=True)
    v_out_d = nc.declare_dram_parameter("v_out", [T, HG * HD], F32R, isOutput=True)

    q16_scr = nc.dram_tensor("q16_scr", [HG * HD, T], BF16)
    k16_scr = nc.dram_tensor("k16_scr", [HG * HD, T], BF16)
    v16_scr = nc.dram_tensor("v16_scr", [T, HG * HD], BF16)
    oT_scr = nc.dram_tensor("oT_scr", [HG * HD, T], BF16)

    with tile.TileContext(nc) as tc:
        with contextlib.ExitStack() as xstack:
            xpool = xstack.enter_context(tc.tile_pool(name="xpool", bufs=1))
            x_sb = xpool.tile([128, CB, T], BF16, tag="x")
            for ts in range(TS):
                for cb in range(CB):
                    nc.sync.dma_start(
                        out=x_sb[:, cb, ts * 512:(ts + 1) * 512],
                        in_=xT_d[cb * 128:(cb + 1) * 128, ts * 512:(ts + 1) * 512],
                    )

            # ---------------- phase V: v = x @ Wv.T (natural layout) -------
            with contextlib.ExitStack() as ph:
                wvpool = ph.enter_context(tc.tile_pool(name="wvpool", bufs=1))
                psv = ph.enter_context(tc.tile_pool(name="psv", bufs=3, space="PSUM"))
                vev = ph.enter_context(tc.tile_pool(name="vev", bufs=3))
                for vs in range(2):
                    wv_sb = wvpool.tile([128, CB, 512], BF16, tag="wv")
                    for cb in range(CB):
                        nc.sync.dma_start(
                            out=wv_sb[:, cb, :],
                            in_=wqkvT_d[cb * 128:(cb + 1) * 128,
                                        2048 + vs * 512:2048 + (vs + 1) * 512],
                        )
                    for tb in range(TB):
                        ps = psv.tile([128, 512], F32, tag="ps")
                        for cb in range(CB):
                            nc.tensor.matmul(
                                ps[:],
                                (x_sb[:, cb, tb * 128:(tb + 1) * 128]),
                                (wv_sb[:, cb, :]),
                                start=(cb == 0), stop=(cb == CB - 1),
                            )
                        vt = vev.tile([128, 512], F32R, tag="vout")
                        nc.scalar.copy(out=vt[:], in_=ps[:])
                        nc.sync.dma_start(
                            out=v_out_d[tb * 128:(tb + 1) * 128,
                                        vs * 512:(vs + 1) * 512],
                            in_=vt[:])
                        vt16 = vev.tile([128, 512], BF16, tag="vout16")
                        nc.vector.tensor_copy(out=vt16[:], in_=ps[:])
                        nc.sync.dma_start(
                            out=v16_scr[tb * 128:(tb + 1) * 128,
                                        vs * 512:(vs + 1) * 512],
                            in_=vt16[:])

            # ---------------- phase QK: qT,kT + rope ----------------------
            with contextlib.ExitStack() as ph:
                trig = ph.enter_context(tc.tile_pool(name="trig", bufs=1))
                cos_sb = trig.tile([HD, T], F32R, tag="cos")
                sin_sb = trig.tile([HD, T], F32R, tag="sin")
                nc.sync.dma_start(out=cos_sb[:], in_=cosT_d[:])
                nc.sync.dma_start(out=sin_sb[:], in_=sinT_d[:])

                wpool = ph.enter_context(tc.tile_pool(name="wpool", bufs=2))
                psqk = ph.enter_context(
                    tc.tile_pool(name="psqk", bufs=3, space="PSUM"))
                rpool = ph.enter_context(tc.tile_pool(name="rpool", bufs=3))

                for jb in range(2 * HG):
                    w_sb = wpool.tile([128, CB, 128], BF16, tag="wqk")
                    nc.sync.dma_start(
                        out=w_sb[:],
                        in_=wqkvT_d[:, jb * 128:(jb + 1) * 128].rearrange(
                            "(cb c) j -> c cb j", c=128),
                    )
                    for ts in range(TS):
                        sl = slice(ts * 512, (ts + 1) * 512)
                        ps = psqk.tile([128, 512], F32, tag="ps")
                        for cb in range(CB):
                            nc.tensor.matmul(
                                ps[:], (w_sb[:, cb, :]), (x_sb[:, cb, sl]),
                                start=(cb == 0), stop=(cb == CB - 1),
                            )
                        tmp = rpool.tile([128, 512], F32R, tag="tmp")
                        nc.scalar.copy(out=tmp[:], in_=ps[:])
                        rot = rpool.tile([128, 512], F32R, tag="rot")
                        nc.sync.dma_start(out=rot[0:64, :], in_=tmp[64:128, :])
                        nc.sync.dma_start(out=rot[64:128, :], in_=tmp[0:64, :])
                        nc.vector.tensor_mul(out=tmp[:], in0=tmp[:], in1=cos_sb[:, sl])
                        nc.vector.tensor_mul(out=rot[:], in0=rot[:], in1=sin_sb[:, sl])
                        nc.vector.tensor_add(out=tmp[:], in0=tmp[:], in1=rot[:])
                        if jb >= HG:
                            nc.sync.dma_start(
                                out=kT_out_d[(jb % HG) * 128:(jb % HG + 1) * 128, sl],
                                in_=tmp[:])
                        tmp16 = rpool.tile([128, 512], BF16, tag="tmp16")
                        nc.vector.tensor_copy(out=tmp16[:], in_=tmp[:])
                        dst16 = q16_scr if jb < HG else k16_scr
                        nc.sync.dma_start(
                            out=dst16[(jb % HG) * 128:(jb % HG + 1) * 128, sl],
                            in_=tmp16[:])

        # ---------------- phase A: attention per head ----------------------
        # Transposed-score formulation: S_T[j, i] = k_j.T @ q_i needs no
        # P transposes -- v_sb [j, d] is directly the PV lhsT. Softmax
        # denominators come from an all-ones [128,128] stationary matmul,
        # which also broadcasts them across partitions for free.
        with contextlib.ExitStack() as ph:
            small = ph.enter_context(tc.tile_pool(name="small", bufs=1))
            mask_sb = small.tile([128, 128], F32, tag="mask")
            nc.sync.dma_start(out=mask_sb[:], in_=mask_d[:])
            ones16 = small.tile([128, 128], BF16, tag="ones16")
            nc.vector.memset(ones16[:], 1.0)

            qkpool = ph.enter_context(tc.tile_pool(name="qkpool", bufs=2))
            vpool = ph.enter_context(tc.tile_pool(name="vpool", bufs=2))
            stpool = ph.enter_context(tc.tile_pool(name="stpool", bufs=4))
            otpool = ph.enter_context(tc.tile_pool(name="otpool", bufs=2))
            stats = ph.enter_context(tc.tile_pool(name="stats", bufs=3))
            psS = ph.enter_context(tc.tile_pool(name="psS", bufs=3, space="PSUM"))
            psO = ph.enter_context(tc.tile_pool(name="psO", bufs=2, space="PSUM"))
            psSum = ph.enter_context(tc.tile_pool(name="psSum", bufs=2, space="PSUM"))

            for h in range(HG):
                hsl = slice(h * 128, (h + 1) * 128)
                q_sb = qkpool.tile([128, T], BF16, tag="q")
                nc.sync.dma_start(out=q_sb[:], in_=q16_scr[hsl, :])
                k_sb = qkpool.tile([128, T], BF16, tag="k")
                nc.sync.dma_start(out=k_sb[:], in_=k16_scr[hsl, :])
                v_sb = vpool.tile([128, TB, 128], BF16, tag="v")
                nc.sync.dma_start(
                    out=v_sb[:],
                    in_=v16_scr[:, hsl].rearrange("(tb t) d -> t tb d", t=128))
                oT_sb = otpool.tile([128, T], BF16, tag="oT")

                for ig in range(4):
                    igsl = slice(ig * 512, (ig + 1) * 512)
                    njb = 4 * ig + 4
                    po = psO.tile([128, 512], F32, tag="psO")
                    sums = psSum.tile([128, 512], F32, tag="sums")

                    def s_exp(jb):
                        # S_T block -> causal mask on diag -> exp -> bf16 P_T.
                        # In-group blocks only compute the live columns; the
                        # fully-masked prefix of P_T is zeroed instead.
                        p = jb - 4 * ig
                        w0 = max(p, 0) * 128          # first live column
                        ps = psS.tile([128, 512], F32, tag="psS")
                        nc.tensor.matmul(
                            ps[:, w0:],
                            (k_sb[:, jb * 128:(jb + 1) * 128]),
                            (q_sb[:, ig * 512 + w0:(ig + 1) * 512]),
                            start=True, stop=True,
                        )
                        if p >= 0:
                            nc.vector.tensor_add(
                                out=ps[:, w0:w0 + 128],
                                in0=ps[:, w0:w0 + 128], in1=mask_sb[:])
                        s16t = stpool.tile([128, 512], BF16, tag="s16t")
                        if w0 > 0:
                            nc.vector.memset(s16t[:, :w0], 0.0)
                        nc.scalar.activation(
                            out=s16t[:, w0:], in_=ps[:, w0:],
                            func=mybir.ActivationFunctionType.Exp,
                            bias=0.0, scale=SCALE)
                        return s16t

                    def sum_pv(jb, s16t):
                        nc.tensor.matmul(
                            sums[:], (ones16[:]), (s16t[:]),
                            start=(jb == 0), stop=(jb == njb - 1),
                        )
                        nc.tensor.matmul(
                            po[:], (v_sb[:, jb, :]), (s16t[:]),
                            start=(jb == 0), stop=(jb == njb - 1),
                        )

                    # software pipeline: S/exp of jb+1 ahead of sum/PV of jb
                    prev = s_exp(0)
                    for jb in range(1, njb):
                        cur = s_exp(jb)
                        sum_pv(jb - 1, prev)
                        prev = cur
                    sum_pv(njb - 1, prev)

                    rs = stats.tile([128, 512], F32, tag="rs")
                    nc.vector.reciprocal(out=rs[:], in_=sums[:])
                    nc.vector.tensor_mul(
                        out=oT_sb[:, igsl], in0=po[:], in1=rs[:])
                nc.sync.dma_start(out=oT_scr[hsl, :], in_=oT_sb[:])

        # ---------------- phase O: out projection --------------------------
        with contextlib.ExitStack() as ph:
            wopool = ph.enter_context(tc.tile_pool(name="wopool", bufs=1))
            oipool = ph.enter_context(tc.tile_pool(name="oipool", bufs=1))
            psP = ph.enter_context(tc.tile_pool(name="psP", bufs=3, space="PSUM"))
            fpool = ph.enter_context(tc.tile_pool(name="fpool", bufs=3))

            wo_sb = wopool.tile([128, HG, C], BF16, tag="wo")
            oi_sb = oipool.tile([128, HG, T], BF16, tag="oi")
            for h in range(HG):
                nc.sync.dma_start(
                    out=wo_sb[:, h, :], in_=w_outT_d[h * 128:(h + 1) * 128, :])
                nc.sync.dma_start(
                    out=oi_sb[:, h, :], in_=oT_scr[h * 128:(h + 1) * 128, :])

            for tb in range(TB):
                for os_ in range(TS):
                    osl = slice(os_ * 512, (os_ + 1) * 512)
                    ps = psP.tile([128, 512], F32, tag="psP")
                    for h in range(HG):
                        nc.tensor.matmul(
                            ps[:],
                            (oi_sb[:, h, tb * 128:(tb + 1) * 128]),
                            (wo_sb[:, h, osl]),
                            start=(h == 0), stop=(h == HG - 1),
                        )
                    ft = fpool.tile([128, 512], F32, tag="fout")
                    nc.scalar.copy(out=ft[:], in_=ps[:])
                    nc.sync.dma_start(
                        out=outp_d[tb * 128:(tb + 1) * 128, osl], in_=ft[:])

    nc.finalize()
    return nc


_NC_CACHE = None


def _host_tables():
    inv_freq = 1.0 / (THETA ** (np.arange(0, HD, 2, dtype=np.float64) / HD))
    t_ar = np.arange(T, dtype=np.float64)
    emb = np.concatenate([np.outer(t_ar, inv_freq)] * 2, axis=-1)   # [T, 128]
    cosT = np.cos(emb).T.astype(np.float32).copy()
    sinT = np.sin(emb).T.astype(np.float32).copy()
    sinT[:64] *= -1.0
    # transposed causal mask for an S_T diagonal block: mask[j, i] kills
    # keys j > query i
    jj = np.arange(128)[:, None]
    ii = np.arange(128)[None, :]
    mask = np.where(jj > ii, np.float32(-1e30), np.float32(0.0))
    return cosT, sinT, mask


def kernel(x, wqkv, w_out):
    global _NC_CACHE, LAST_RESULTS
    x = np.ascontiguousarray(np.asarray(x, dtype=np.float32))
    wqkv = np.asarray(wqkv, dtype=np.float32)
    w_out = np.asarray(w_out, dtype=np.float32)

    if _NC_CACHE is None:
        _NC_CACHE = build_nc()
    nc = _NC_CACHE

    cosT, sinT, mask = _host_tables()
    in_maps = []
    for core in range(8):
        b, g = core // 2, core % 2
        rows = slice(g * HG * HD, (g + 1) * HG * HD)
        wq = wqkv[0 * C:1 * C][rows]
        wk = wqkv[1 * C:2 * C][rows]
        wv = wqkv[2 * C:3 * C][rows]
        in_maps.append({
            "xT": np.ascontiguousarray(x[b].T).astype(ml_dtypes.bfloat16),
            "wqkvT": np.ascontiguousarray(
                np.concatenate([wq, wk, wv], axis=0).T).astype(ml_dtypes.bfloat16),
            "w_outT": np.ascontiguousarray(
                w_out[:, rows].T).astype(ml_dtypes.bfloat16),
            "cosT": cosT,
            "sinT": sinT,
            "mask": mask,
        })

    trace = bool(os.environ.get("KERNEL_TRACE"))
    if trace:
        _install_ntff_hook()
    res = run_bass_kernel_spmd(nc, in_maps, list(range(8)), trace=trace)
    LAST_RESULTS = res

    out = np.zeros((B, T, C), np.float32)
    k_full = np.empty((B, H, T, HD), np.float32)
    v_full = np.empty((B, H, T, HD), np.float32)
    for core in range(8):
        b, g = core // 2, core % 2
        r = res.results[core]
        out[b] += r["outp"]
        k_full[b, g * HG:(g + 1) * HG] = (
            r["kT_out"].reshape(HG, HD, T).transpose(0, 2, 1))
        v_full[b, g * HG:(g + 1) * HG] = (
            r["v_out"].reshape(T, HG, HD).transpose(1, 0, 2))
    return out, k_full, v_full


# revision 11
# speedup vs baseline: 1.1807x; 1.1046x over previous
"""Causal self-attention (B=4, T=2048, C=2048, H=16, rope) on 8 TRN2 NeuronCores.

Sharding: core = (batch b, head-group g) with b in 0..3, g in 0..1. Each core
owns 8 heads of one batch: computes its qkv projection shard, rope, causal
attention, and a partial out-projection (contracting only its 1024 columns of
the 2048-dim head-concat axis). Host sums the two partials per batch
(the "all-reduce after out_proj"), and reassembles k (post-rope) and v.

Per-core dataflow (all matmuls in fp32r on the PE):
  xT [C,T] resident in SBUF.
  v [T,1024] produced in natural layout (lhsT = xT block), written to the v
    output and re-read per head during attention.
  qT,kT [1024,T] produced head-transposed (lhsT = wqkvT panel), rope applied
    at psum-evict time (partition-half swap via SBUF-SBUF DMA), spilled to
    DRAM (the kT spill doubles as the k output).
  Attention per head: S = qT_blk.T @ kT chunks -> causal mask on diag block ->
    softmax (max on DVE, exp+row-sum fused on ACT, 1/sum applied to P) ->
    PE-transpose P blocks -> oT accum = v_blk.T @ P_T, spilled per head.
  out_proj: out[t,o] accumulated over 8 heads (lhsT = oT block, rhs = w_outT).
"""

import os
import sys
import types
import contextlib

sys.path.insert(0, "/opt/trn_rl_repo")

import numpy as np
import ml_dtypes

import concourse.bacc as bacc
import concourse.mybir as mybir
import concourse.tile as tile
from concourse.bass_utils import run_bass_kernel_spmd

F32 = mybir.dt.float32
F32R = mybir.dt.float32r
BF16 = mybir.dt.bfloat16

B, T, C = 4, 2048, 2048
H, HD = 16, 128
HG = 8                      # heads per core
CB = C // 128               # 16 contraction blocks
TB = T // 128               # 16 time blocks
TS = T // 512               # 4 time supertiles
SCALE = 1.0 / np.sqrt(HD)
THETA = 10000.0

LAST_RESULTS = None


def _install_ntff_hook():
    """Make run_bass_kernel_spmd(trace=True) work: register the axon NTFF
    profile hook that the image's antenv package lacks."""
    if "antenv.axon_hooks" in sys.modules:
        return True
    try:
        sys.path.insert(0, "/root/.axon_site")
        from trn_agent_boot.trn_boot import _ntff_profile_via_ctypes
        hook = _ntff_profile_via_ctypes("/opt/axon/libaxon_pjrt.so")
    except Exception:
        return False
    if hook is None:
        return False
    mod = types.ModuleType("antenv.axon_hooks")
    mod._hook = hook
    mod.get_axon_ntff_profile_hook = lambda: mod._hook
    mod.set_axon_ntff_profile_hook = lambda h: setattr(mod, "_hook", h)
    sys.modules["antenv.axon_hooks"] = mod
    import antenv
    antenv.axon_hooks = mod
    return True


def _r(ap):
    return ap.bitcast(F32R)


def build_nc():
    nc = bacc.Bacc(None, target_bir_lowering=False)

    xT_d = nc.declare_dram_parameter("xT", [C, T], BF16, isOutput=False)
    wqkvT_d = nc.declare_dram_parameter("wqkvT", [C, 3 * HG * HD], BF16, isOutput=False)
    w_outT_d = nc.declare_dram_parameter("w_outT", [HG * HD, C], BF16, isOutput=False)
    cosT_d = nc.declare_dram_parameter("cosT", [HD, T], F32, isOutput=False)
    sinT_d = nc.declare_dram_parameter("sinT", [HD, T], F32, isOutput=False)
    mask_d = nc.declare_dram_parameter("mask", [128, 128], F32, isOutput=False)

    outp_d = nc.declare_dram_parameter("outp", [T, C], F32, isOutput=True)
    kT_out_d = nc.declare_dram_parameter("kT_out", [HG * HD, T], F32, isOutput=True)
    v_out_d = nc.declare_dram_parameter("v_out", [T, HG * HD], F32, isOutput=True)

    q16_scr = nc.dram_tensor("q16_scr", [HG * HD, T], BF16)
    k16_scr = nc.dram_tensor("k16_scr", [HG * HD, T], BF16)
    v16_scr = nc.dram_tensor("v16_scr", [T, HG * HD], BF16)

    with tile.TileContext(nc) as tc:
        with contextlib.ExitStack() as xstack:
            xpool = xstack.enter_context(tc.tile_pool(name="xpool", bufs=1))
            x_sb = xpool.tile([128, CB, T], BF16, tag="x")
            for ts in range(TS):
                for cb in range(CB):
                    nc.sync.dma_start(
                        out=x_sb[:, cb, ts * 512:(ts + 1) * 512],
                        in_=xT_d[cb * 128:(cb + 1) * 128, ts * 512:(ts + 1) * 512],
                    )

            # ---------------- phase V: v = x @ Wv.T (natural layout) -------
            with contextlib.ExitStack() as ph:
                wvpool = ph.enter_context(tc.tile_pool(name="wvpool", bufs=1))
                psv = ph.enter_context(tc.tile_pool(name="psv", bufs=3, space="PSUM"))
                vev = ph.enter_context(tc.tile_pool(name="vev", bufs=3))
                for vs in range(2):
                    wv_sb = wvpool.tile([128, CB, 512], BF16, tag="wv")
                    for cb in range(CB):
                        nc.sync.dma_start(
                            out=wv_sb[:, cb, :],
                            in_=wqkvT_d[cb * 128:(cb + 1) * 128,
                                        2048 + vs * 512:2048 + (vs + 1) * 512],
                        )
                    for tb in range(TB):
                        ps = psv.tile([128, 512], F32, tag="ps")
                        for cb in range(CB):
                            nc.tensor.matmul(
                                ps[:],
                                (x_sb[:, cb, tb * 128:(tb + 1) * 128]),
                                (wv_sb[:, cb, :]),
                                start=(cb == 0), stop=(cb == CB - 1),
                            )
                        vt = vev.tile([128, 512], F32, tag="vout")
                        nc.scalar.copy(out=vt[:], in_=ps[:])
                        nc.sync.dma_start(
                            out=v_out_d[tb * 128:(tb + 1) * 128,
                                        vs * 512:(vs + 1) * 512],
                            in_=vt[:])
                        vt16 = vev.tile([128, 512], BF16, tag="vout16")
                        nc.vector.tensor_copy(out=vt16[:], in_=ps[:])
                        nc.sync.dma_start(
                            out=v16_scr[tb * 128:(tb + 1) * 128,
                                        vs * 512:(vs + 1) * 512],
                            in_=vt16[:])

            # ---------------- phase QK: qT,kT + rope ----------------------
            with contextlib.ExitStack() as ph:
                trig = ph.enter_context(tc.tile_pool(name="trig", bufs=1))
                cos_sb = trig.tile([HD, T], F32, tag="cos")
                sin_sb = trig.tile([HD, T], F32, tag="sin")
                nc.sync.dma_start(out=cos_sb[:], in_=cosT_d[:])
                nc.sync.dma_start(out=sin_sb[:], in_=sinT_d[:])

                wpool = ph.enter_context(tc.tile_pool(name="wpool", bufs=2))
                psqk = ph.enter_context(
                    tc.tile_pool(name="psqk", bufs=3, space="PSUM"))
                rpool = ph.enter_context(tc.tile_pool(name="rpool", bufs=3))

                for jb in range(2 * HG):
                    w_sb = wpool.tile([128, CB, 128], BF16, tag="wqk")
                    nc.sync.dma_start(
                        out=w_sb[:],
                        in_=wqkvT_d[:, jb * 128:(jb + 1) * 128].rearrange(
                            "(cb c) j -> c cb j", c=128),
                    )
                    for ts in range(TS):
                        sl = slice(ts * 512, (ts + 1) * 512)
                        ps = psqk.tile([128, 512], F32, tag="ps")
                        for cb in range(CB):
                            nc.tensor.matmul(
                                ps[:], (w_sb[:, cb, :]), (x_sb[:, cb, sl]),
                                start=(cb == 0), stop=(cb == CB - 1),
                            )
                        tmp = rpool.tile([128, 512], F32, tag="tmp")
                        nc.scalar.copy(out=tmp[:], in_=ps[:])
                        rot = rpool.tile([128, 512], F32, tag="rot")
                        nc.sync.dma_start(out=rot[0:64, :], in_=tmp[64:128, :])
                        nc.sync.dma_start(out=rot[64:128, :], in_=tmp[0:64, :])
                        nc.vector.tensor_mul(out=tmp[:], in0=tmp[:], in1=cos_sb[:, sl])
                        nc.vector.tensor_mul(out=rot[:], in0=rot[:], in1=sin_sb[:, sl])
                        nc.vector.tensor_add(out=tmp[:], in0=tmp[:], in1=rot[:])
                        if jb >= HG:
                            nc.sync.dma_start(
                                out=kT_out_d[(jb % HG) * 128:(jb % HG + 1) * 128, sl],
                                in_=tmp[:])
                        tmp16 = rpool.tile([128, 512], BF16, tag="tmp16")
                        nc.vector.tensor_copy(out=tmp16[:], in_=tmp[:])
                        dst16 = q16_scr if jb < HG else k16_scr
                        nc.sync.dma_start(
                            out=dst16[(jb % HG) * 128:(jb % HG + 1) * 128, sl],
                            in_=tmp16[:])

        # ---- attention + out_proj share the oT / w_out SBUF residency ----
        shared = contextlib.ExitStack()
        oipool = shared.enter_context(tc.tile_pool(name="oipool", bufs=1))
        wopool = shared.enter_context(tc.tile_pool(name="wopool", bufs=1))
        oi_sb = oipool.tile([128, HG, T], BF16, tag="oi")
        wo_sb = wopool.tile([128, HG, C], BF16, tag="wo")
        for h in range(HG):
            nc.sync.dma_start(
                out=wo_sb[:, h, :], in_=w_outT_d[h * 128:(h + 1) * 128, :])

        # ---------------- phase A: attention per head ----------------------
        # Transposed-score formulation: S_T[j, i] = k_j.T @ q_i needs no
        # P transposes -- v_sb [j, d] is directly the PV lhsT. Softmax
        # denominators come from an all-ones [128,128] stationary matmul,
        # which also broadcasts them across partitions for free.
        with contextlib.ExitStack() as ph:
            small = ph.enter_context(tc.tile_pool(name="small", bufs=1))
            mask_sb = small.tile([128, 128], F32, tag="mask")
            nc.sync.dma_start(out=mask_sb[:], in_=mask_d[:])
            ones16 = small.tile([128, 128], BF16, tag="ones16")
            nc.vector.memset(ones16[:], 1.0)

            qkpool = ph.enter_context(tc.tile_pool(name="qkpool", bufs=2))
            vpool = ph.enter_context(tc.tile_pool(name="vpool", bufs=2))
            stpool = ph.enter_context(tc.tile_pool(name="stpool", bufs=5))
            stats = ph.enter_context(tc.tile_pool(name="stats", bufs=3))
            psS = ph.enter_context(tc.tile_pool(name="psS", bufs=4, space="PSUM"))
            psO = ph.enter_context(tc.tile_pool(name="psO", bufs=2, space="PSUM"))
            psSum = ph.enter_context(tc.tile_pool(name="psSum", bufs=2, space="PSUM"))

            for h in range(HG):
                hsl = slice(h * 128, (h + 1) * 128)
                q_sb = qkpool.tile([128, T], BF16, tag="q")
                nc.sync.dma_start(out=q_sb[:], in_=q16_scr[hsl, :])
                k_sb = qkpool.tile([128, T], BF16, tag="k")
                nc.sync.dma_start(out=k_sb[:], in_=k16_scr[hsl, :])
                v_sb = vpool.tile([128, TB, 128], BF16, tag="v")
                nc.sync.dma_start(
                    out=v_sb[:],
                    in_=v16_scr[:, hsl].rearrange("(tb t) d -> t tb d", t=128))

                for ig in range(4):
                    igsl = slice(ig * 512, (ig + 1) * 512)
                    njb = 4 * ig + 4
                    po = psO.tile([128, 512], F32, tag="psO")
                    sums = psSum.tile([128, 512], F32, tag="sums")

                    def s_exp(jb):
                        # S_T block -> causal mask on diag -> exp -> bf16 P_T.
                        # In-group blocks only compute the live columns; the
                        # fully-masked prefix of P_T is zeroed instead.
                        p = jb - 4 * ig
                        w0 = max(p, 0) * 128          # first live column
                        ps = psS.tile([128, 512], F32, tag="psS")
                        nc.tensor.matmul(
                            ps[:, w0:],
                            (k_sb[:, jb * 128:(jb + 1) * 128]),
                            (q_sb[:, ig * 512 + w0:(ig + 1) * 512]),
                            start=True, stop=True,
                        )
                        if p >= 0:
                            nc.vector.tensor_add(
                                out=ps[:, w0:w0 + 128],
                                in0=ps[:, w0:w0 + 128], in1=mask_sb[:])
                        s16t = stpool.tile([128, 512], BF16, tag="s16t")
                        if w0 > 0:
                            nc.vector.memset(s16t[:, :w0], 0.0)
                        nc.scalar.activation(
                            out=s16t[:, w0:], in_=ps[:, w0:],
                            func=mybir.ActivationFunctionType.Exp,
                            bias=0.0, scale=SCALE)
                        return s16t

                    def sum_pv(jb, s16t):
                        nc.tensor.matmul(
                            sums[:], (ones16[:]), (s16t[:]),
                            start=(jb == 0), stop=(jb == njb - 1),
                        )
                        nc.tensor.matmul(
                            po[:], (v_sb[:, jb, :]), (s16t[:]),
                            start=(jb == 0), stop=(jb == njb - 1),
                        )

                    # 2-deep software pipeline: S/exp of jb+1 and jb+2 sit
                    # ahead of sum/PV of jb in the PE queue, covering the
                    # exp latency.
                    win = [s_exp(0)]
                    if njb > 1:
                        win.append(s_exp(1))
                    for jb in range(2, njb):
                        win.append(s_exp(jb))
                        sum_pv(jb - 2, win.pop(0))
                    if njb > 1:
                        sum_pv(njb - 2, win.pop(0))
                    sum_pv(njb - 1, win.pop(0))

                    rs = stats.tile([128, 512], F32, tag="rs")
                    nc.vector.reciprocal_approx_fast(out=rs[:], in_=sums[:])
                    nc.vector.tensor_mul(
                        out=oi_sb[:, h, igsl], in0=po[:], in1=rs[:])

        # ---------------- phase O: out projection --------------------------
        with contextlib.ExitStack() as ph:
            psP = ph.enter_context(tc.tile_pool(name="psP", bufs=3, space="PSUM"))
            fpool = ph.enter_context(tc.tile_pool(name="fpool", bufs=3))

            for tb in range(TB):
                for os_ in range(TS):
                    osl = slice(os_ * 512, (os_ + 1) * 512)
                    ps = psP.tile([128, 512], F32, tag="psP")
                    for h in range(HG):
                        nc.tensor.matmul(
                            ps[:],
                            (oi_sb[:, h, tb * 128:(tb + 1) * 128]),
                            (wo_sb[:, h, osl]),
                            start=(h == 0), stop=(h == HG - 1),
                        )
                    ft = fpool.tile([128, 512], F32, tag="fout")
                    nc.scalar.copy(out=ft[:], in_=ps[:])
                    nc.sync.dma_start(
                        out=outp_d[tb * 128:(tb + 1) * 128, osl], in_=ft[:])
        shared.close()

    nc.finalize()
    return nc


_NC_CACHE = None


def _host_tables():
    inv_freq = 1.0 / (THETA ** (np.arange(0, HD, 2, dtype=np.float64) / HD))
    t_ar = np.arange(T, dtype=np.float64)
    emb = np.concatenate([np.outer(t_ar, inv_freq)] * 2, axis=-1)   # [T, 128]
    cosT = np.cos(emb).T.astype(np.float32).copy()
    sinT = np.sin(emb).T.astype(np.float32).copy()
    sinT[:64] *= -1.0
    # transposed causal mask for an S_T diagonal block: mask[j, i] kills
    # keys j > query i
    jj = np.arange(128)[:, None]
    ii = np.arange(128)[None, :]
    mask = np.where(jj > ii, np.float32(-1e30), np.float32(0.0))
    return cosT, sinT, mask


def kernel(x, wqkv, w_out):
    global _NC_CACHE, LAST_RESULTS
    x = np.ascontiguousarray(np.asarray(x, dtype=np.float32))
    wqkv = np.asarray(wqkv, dtype=np.float32)
    w_out = np.asarray(w_out, dtype=np.float32)

    if _NC_CACHE is None:
        _NC_CACHE = build_nc()
    nc = _NC_CACHE

    cosT, sinT, mask = _host_tables()
    in_maps = []
    for core in range(8):
        b, g = core // 2, core % 2
        rows = slice(g * HG * HD, (g + 1) * HG * HD)
        wq = wqkv[0 * C:1 * C][rows]
        wk = wqkv[1 * C:2 * C][rows]
        wv = wqkv[2 * C:3 * C][rows]
        in_maps.append({
            "xT": np.ascontiguousarray(x[b].T).astype(ml_dtypes.bfloat16),
            "wqkvT": np.ascontiguousarray(
                np.concatenate([wq, wk, wv], axis=0).T).astype(ml_dtypes.bfloat16),
            "w_outT": np.ascontiguousarray(
                w_out[:, rows].T).astype(ml_dtypes.bfloat16),
            "cosT": cosT,
            "sinT": sinT,
            "mask": mask,
        })

    trace = bool(os.environ.get("KERNEL_TRACE"))
    if trace:
        _install_ntff_hook()
    res = run_bass_kernel_spmd(nc, in_maps, list(range(8)), trace=trace)
    LAST_RESULTS = res

    out = np.zeros((B, T, C), np.float32)
    k_full = np.empty((B, H, T, HD), np.float32)
    v_full = np.empty((B, H, T, HD), np.float32)
    for core in range(8):
        b, g = core // 2, core % 2
        r = res.results[core]
        out[b] += r["outp"]
        k_full[b, g * HG:(g + 1) * HG] = (
            r["kT_out"].reshape(HG, HD, T).transpose(0, 2, 1))
        v_full[b, g * HG:(g + 1) * HG] = (
            r["v_out"].reshape(T, HG, HD).transpose(1, 0, 2))
    return out, k_full, v_full


# revision 13
# speedup vs baseline: 1.1886x; 1.0067x over previous
"""Causal self-attention (B=4, T=2048, C=2048, H=16, rope) on 8 TRN2 NeuronCores.

Sharding: core = (batch b, head-group g) with b in 0..3, g in 0..1. Each core
owns 8 heads of one batch: computes its qkv projection shard, rope, causal
attention, and a partial out-projection (contracting only its 1024 columns of
the 2048-dim head-concat axis). Host sums the two partials per batch
(the "all-reduce after out_proj"), and reassembles k (post-rope) and v.

Per-core dataflow (all matmuls in fp32r on the PE):
  xT [C,T] resident in SBUF.
  v [T,1024] produced in natural layout (lhsT = xT block), written to the v
    output and re-read per head during attention.
  qT,kT [1024,T] produced head-transposed (lhsT = wqkvT panel), rope applied
    at psum-evict time (partition-half swap via SBUF-SBUF DMA), spilled to
    DRAM (the kT spill doubles as the k output).
  Attention per head: S = qT_blk.T @ kT chunks -> causal mask on diag block ->
    softmax (max on DVE, exp+row-sum fused on ACT, 1/sum applied to P) ->
    PE-transpose P blocks -> oT accum = v_blk.T @ P_T, spilled per head.
  out_proj: out[t,o] accumulated over 8 heads (lhsT = oT block, rhs = w_outT).
"""

import os
import sys
import types
import contextlib

sys.path.insert(0, "/opt/trn_rl_repo")

import numpy as np
import ml_dtypes

import concourse.bacc as bacc
import concourse.mybir as mybir
import concourse.tile as tile
from concourse.bass_utils import run_bass_kernel_spmd

F32 = mybir.dt.float32
F32R = mybir.dt.float32r
BF16 = mybir.dt.bfloat16

B, T, C = 4, 2048, 2048
H, HD = 16, 128
HG = 8                      # heads per core
CB = C // 128               # 16 contraction blocks
TB = T // 128               # 16 time blocks
TS = T // 512               # 4 time supertiles
SCALE = 1.0 / np.sqrt(HD)
THETA = 10000.0

LAST_RESULTS = None


def _install_ntff_hook():
    """Make run_bass_kernel_spmd(trace=True) work: register the axon NTFF
    profile hook that the image's antenv package lacks."""
    if "antenv.axon_hooks" in sys.modules:
        return True
    try:
        sys.path.insert(0, "/root/.axon_site")
        from trn_agent_boot.trn_boot import _ntff_profile_via_ctypes
        hook = _ntff_profile_via_ctypes("/opt/axon/libaxon_pjrt.so")
    except Exception:
        return False
    if hook is None:
        return False
    mod = types.ModuleType("antenv.axon_hooks")
    mod._hook = hook
    mod.get_axon_ntff_profile_hook = lambda: mod._hook
    mod.set_axon_ntff_profile_hook = lambda h: setattr(mod, "_hook", h)
    sys.modules["antenv.axon_hooks"] = mod
    import antenv
    antenv.axon_hooks = mod
    return True


def _r(ap):
    return ap.bitcast(F32R)


def build_nc():
    nc = bacc.Bacc(None, target_bir_lowering=False)

    xT_d = nc.declare_dram_parameter("xT", [C, T], BF16, isOutput=False)
    wqkvT_d = nc.declare_dram_parameter("wqkvT", [C, 3 * HG * HD], BF16, isOutput=False)
    w_outT_d = nc.declare_dram_parameter("w_outT", [HG * HD, C], BF16, isOutput=False)
    cosT_d = nc.declare_dram_parameter("cosT", [HD, T], F32, isOutput=False)
    sinT_d = nc.declare_dram_parameter("sinT", [HD, T], F32, isOutput=False)
    mask_d = nc.declare_dram_parameter("mask", [128, 128], F32, isOutput=False)

    outp_d = nc.declare_dram_parameter("outp", [T, C], F32, isOutput=True)
    kT_out_d = nc.declare_dram_parameter("kT_out", [HG * HD, T], F32, isOutput=True)
    v_out_d = nc.declare_dram_parameter("v_out", [T, HG * HD], F32, isOutput=True)

    v16_scr = nc.dram_tensor("v16_scr", [T, HG * HD], BF16)

    with tile.TileContext(nc) as tc:
        qkstk = contextlib.ExitStack()
        qk16pool = qkstk.enter_context(tc.tile_pool(name="qk16pool", bufs=1))
        qk16_sb = qk16pool.tile([128, 2 * HG, T], BF16, tag="qk16")
        with contextlib.ExitStack() as xstack:
            xpool = xstack.enter_context(tc.tile_pool(name="xpool", bufs=1))
            x_sb = xpool.tile([128, CB, T], BF16, tag="x")
            for ts in range(TS):
                for cb in range(CB):
                    nc.sync.dma_start(
                        out=x_sb[:, cb, ts * 512:(ts + 1) * 512],
                        in_=xT_d[cb * 128:(cb + 1) * 128, ts * 512:(ts + 1) * 512],
                    )

            # ---------------- phase V: v = x @ Wv.T (natural layout) -------
            with contextlib.ExitStack() as ph:
                wvpool = ph.enter_context(tc.tile_pool(name="wvpool", bufs=1))
                psv = ph.enter_context(tc.tile_pool(name="psv", bufs=3, space="PSUM"))
                vev = ph.enter_context(tc.tile_pool(name="vev", bufs=3))
                for vs in range(2):
                    wv_sb = wvpool.tile([128, CB, 512], BF16, tag="wv")
                    for cb in range(CB):
                        nc.sync.dma_start(
                            out=wv_sb[:, cb, :],
                            in_=wqkvT_d[cb * 128:(cb + 1) * 128,
                                        2048 + vs * 512:2048 + (vs + 1) * 512],
                        )
                    for tb in range(TB):
                        ps = psv.tile([128, 512], F32, tag="ps")
                        for cb in range(CB):
                            nc.tensor.matmul(
                                ps[:],
                                (x_sb[:, cb, tb * 128:(tb + 1) * 128]),
                                (wv_sb[:, cb, :]),
                                start=(cb == 0), stop=(cb == CB - 1),
                            )
                        vt = vev.tile([128, 512], F32, tag="vout")
                        nc.scalar.copy(out=vt[:], in_=ps[:])
                        nc.sync.dma_start(
                            out=v_out_d[tb * 128:(tb + 1) * 128,
                                        vs * 512:(vs + 1) * 512],
                            in_=vt[:])
                        vt16 = vev.tile([128, 512], BF16, tag="vout16")
                        nc.vector.tensor_copy(out=vt16[:], in_=ps[:])
                        nc.sync.dma_start(
                            out=v16_scr[tb * 128:(tb + 1) * 128,
                                        vs * 512:(vs + 1) * 512],
                            in_=vt16[:])

            # ---------------- phase QK: qT,kT + rope ----------------------
            with contextlib.ExitStack() as ph:
                trig = ph.enter_context(tc.tile_pool(name="trig", bufs=1))
                cos_sb = trig.tile([HD, T], F32, tag="cos")
                sin_sb = trig.tile([HD, T], F32, tag="sin")
                nc.sync.dma_start(out=cos_sb[:], in_=cosT_d[:])
                nc.sync.dma_start(out=sin_sb[:], in_=sinT_d[:])

                wpool = ph.enter_context(tc.tile_pool(name="wpool", bufs=2))
                psqk = ph.enter_context(
                    tc.tile_pool(name="psqk", bufs=3, space="PSUM"))
                rpool = ph.enter_context(tc.tile_pool(name="rpool", bufs=3))

                for jb in range(2 * HG):
                    w_sb = wpool.tile([128, CB, 128], BF16, tag="wqk")
                    nc.sync.dma_start(
                        out=w_sb[:],
                        in_=wqkvT_d[:, jb * 128:(jb + 1) * 128].rearrange(
                            "(cb c) j -> c cb j", c=128),
                    )
                    for ts in range(TS):
                        sl = slice(ts * 512, (ts + 1) * 512)
                        ps = psqk.tile([128, 512], F32, tag="ps")
                        for cb in range(CB):
                            nc.tensor.matmul(
                                ps[:], (w_sb[:, cb, :]), (x_sb[:, cb, sl]),
                                start=(cb == 0), stop=(cb == CB - 1),
                            )
                        tmp = rpool.tile([128, 512], F32, tag="tmp")
                        nc.scalar.copy(out=tmp[:], in_=ps[:])
                        rot = rpool.tile([128, 512], F32, tag="rot")
                        nc.sync.dma_start(out=rot[0:64, :], in_=tmp[64:128, :])
                        nc.sync.dma_start(out=rot[64:128, :], in_=tmp[0:64, :])
                        nc.vector.tensor_mul(out=tmp[:], in0=tmp[:], in1=cos_sb[:, sl])
                        nc.vector.tensor_mul(out=rot[:], in0=rot[:], in1=sin_sb[:, sl])
                        nc.vector.tensor_add(out=tmp[:], in0=tmp[:], in1=rot[:])
                        if jb >= HG:
                            nc.sync.dma_start(
                                out=kT_out_d[(jb % HG) * 128:(jb % HG + 1) * 128, sl],
                                in_=tmp[:])
                        nc.vector.tensor_copy(
                            out=qk16_sb[:, jb, sl], in_=tmp[:])

        # ---- attention + out_proj share the oT / w_out SBUF residency ----
        shared = contextlib.ExitStack()
        oipool = shared.enter_context(tc.tile_pool(name="oipool", bufs=1))
        wopool = shared.enter_context(tc.tile_pool(name="wopool", bufs=1))
        oi_sb = oipool.tile([128, HG, T], BF16, tag="oi")
        wo_sb = wopool.tile([128, HG, C], BF16, tag="wo")
        for h in range(HG):
            nc.sync.dma_start(
                out=wo_sb[:, h, :], in_=w_outT_d[h * 128:(h + 1) * 128, :])

        # ---------------- phase A: attention per head ----------------------
        # Transposed-score formulation: S_T[j, i] = k_j.T @ q_i needs no
        # P transposes -- v_sb [j, d] is directly the PV lhsT. Softmax
        # denominators come from an all-ones [128,128] stationary matmul,
        # which also broadcasts them across partitions for free.
        with contextlib.ExitStack() as ph:
            small = ph.enter_context(tc.tile_pool(name="small", bufs=1))
            mask_sb = small.tile([128, 128], F32, tag="mask")
            nc.sync.dma_start(out=mask_sb[:], in_=mask_d[:])
            ones16 = small.tile([128, 128], BF16, tag="ones16")
            nc.vector.memset(ones16[:], 1.0)

            vpool = ph.enter_context(tc.tile_pool(name="vpool", bufs=2))
            stpool = ph.enter_context(tc.tile_pool(name="stpool", bufs=5))
            stats = ph.enter_context(tc.tile_pool(name="stats", bufs=3))
            psS = ph.enter_context(tc.tile_pool(name="psS", bufs=4, space="PSUM"))
            psO = ph.enter_context(tc.tile_pool(name="psO", bufs=2, space="PSUM"))
            psSum = ph.enter_context(tc.tile_pool(name="psSum", bufs=2, space="PSUM"))

            for h in range(HG):
                hsl = slice(h * 128, (h + 1) * 128)
                q_sb = qk16_sb[:, h, :]
                k_sb = qk16_sb[:, HG + h, :]
                v_sb = vpool.tile([128, TB, 128], BF16, tag="v")
                nc.sync.dma_start(
                    out=v_sb[:],
                    in_=v16_scr[:, hsl].rearrange("(tb t) d -> t tb d", t=128))

                for ig in range(4):
                    igsl = slice(ig * 512, (ig + 1) * 512)
                    njb = 4 * ig + 4
                    po = psO.tile([128, 512], F32, tag="psO")
                    sums = psSum.tile([128, 512], F32, tag="sums")

                    def s_exp(jb):
                        # S_T block -> causal mask on diag -> exp -> bf16 P_T.
                        # In-group blocks only compute the live columns; the
                        # fully-masked prefix of P_T is zeroed instead.
                        p = jb - 4 * ig
                        w0 = max(p, 0) * 128          # first live column
                        ps = psS.tile([128, 512], F32, tag="psS")
                        nc.tensor.matmul(
                            ps[:, w0:],
                            (k_sb[:, jb * 128:(jb + 1) * 128]),
                            (q_sb[:, ig * 512 + w0:(ig + 1) * 512]),
                            start=True, stop=True,
                        )
                        if p >= 0:
                            nc.vector.tensor_add(
                                out=ps[:, w0:w0 + 128],
                                in0=ps[:, w0:w0 + 128], in1=mask_sb[:])
                        s16t = stpool.tile([128, 512], BF16, tag="s16t")
                        if w0 > 0:
                            nc.vector.memset(s16t[:, :w0], 0.0)
                        nc.scalar.activation(
                            out=s16t[:, w0:], in_=ps[:, w0:],
                            func=mybir.ActivationFunctionType.Exp,
                            bias=0.0, scale=SCALE)
                        return s16t

                    def sum_pv(jb, s16t):
                        nc.tensor.matmul(
                            sums[:], (ones16[:]), (s16t[:]),
                            start=(jb == 0), stop=(jb == njb - 1),
                        )
                        nc.tensor.matmul(
                            po[:], (v_sb[:, jb, :]), (s16t[:]),
                            start=(jb == 0), stop=(jb == njb - 1),
                        )

                    # 2-deep software pipeline: S/exp of jb+1 and jb+2 sit
                    # ahead of sum/PV of jb in the PE queue, covering the
                    # exp latency.
                    win = [s_exp(0)]
                    if njb > 1:
                        win.append(s_exp(1))
                    for jb in range(2, njb):
                        win.append(s_exp(jb))
                        sum_pv(jb - 2, win.pop(0))
                    if njb > 1:
                        sum_pv(njb - 2, win.pop(0))
                    sum_pv(njb - 1, win.pop(0))

                    rs = stats.tile([128, 512], F32, tag="rs")
                    nc.vector.reciprocal_approx_fast(out=rs[:], in_=sums[:])
                    nc.vector.tensor_mul(
                        out=oi_sb[:, h, igsl], in0=po[:], in1=rs[:])

        # ---------------- phase O: out projection --------------------------
        with contextlib.ExitStack() as ph:
            psP = ph.enter_context(tc.tile_pool(name="psP", bufs=3, space="PSUM"))
            fpool = ph.enter_context(tc.tile_pool(name="fpool", bufs=3))

            for tb in range(TB):
                for os_ in range(TS):
                    osl = slice(os_ * 512, (os_ + 1) * 512)
                    ps = psP.tile([128, 512], F32, tag="psP")
                    for h in range(HG):
                        nc.tensor.matmul(
                            ps[:],
                            (oi_sb[:, h, tb * 128:(tb + 1) * 128]),
                            (wo_sb[:, h, osl]),
                            start=(h == 0), stop=(h == HG - 1),
                        )
                    ft = fpool.tile([128, 512], F32, tag="fout")
                    nc.scalar.copy(out=ft[:], in_=ps[:])
                    nc.sync.dma_start(
                        out=outp_d[tb * 128:(tb + 1) * 128, osl], in_=ft[:])
        shared.close()
        qkstk.close()

    nc.finalize()
    return nc


_NC_CACHE = None


def _host_tables():
    inv_freq = 1.0 / (THETA ** (np.arange(0, HD, 2, dtype=np.float64) / HD))
    t_ar = np.arange(T, dtype=np.float64)
    emb = np.concatenate([np.outer(t_ar, inv_freq)] * 2, axis=-1)   # [T, 128]
    cosT = np.cos(emb).T.astype(np.float32).copy()
    sinT = np.sin(emb).T.astype(np.float32).copy()
    sinT[:64] *= -1.0
    # transposed causal mask for an S_T diagonal block: mask[j, i] kills
    # keys j > query i
    jj = np.arange(128)[:, None]
    ii = np.arange(128)[None, :]
    mask = np.where(jj > ii, np.float32(-1e30), np.float32(0.0))
    return cosT, sinT, mask


def kernel(x, wqkv, w_out):
    global _NC_CACHE, LAST_RESULTS
    x = np.ascontiguousarray(np.asarray(x, dtype=np.float32))
    wqkv = np.asarray(wqkv, dtype=np.float32)
    w_out = np.asarray(w_out, dtype=np.float32)

    if _NC_CACHE is None:
        _NC_CACHE = build_nc()
    nc = _NC_CACHE

    cosT, sinT, mask = _host_tables()
    in_maps = []
    for core in range(8):
        b, g = core // 2, core % 2
        rows = slice(g * HG * HD, (g + 1) * HG * HD)
        wq = wqkv[0 * C:1 * C][rows]
        wk = wqkv[1 * C:2 * C][rows]
        wv = wqkv[2 * C:3 * C][rows]
        in_maps.append({
            "xT": np.ascontiguousarray(x[b].T).astype(ml_dtypes.bfloat16),
            "wqkvT": np.ascontiguousarray(
                np.concatenate([wq, wk, wv], axis=0).T).astype(ml_dtypes.bfloat16),
            "w_outT": np.ascontiguousarray(
                w_out[:, rows].T).astype(ml_dtypes.bfloat16),
            "cosT": cosT,
            "sinT": sinT,
            "mask": mask,
        })

    trace = bool(os.environ.get("KERNEL_TRACE"))
    if trace:
        _install_ntff_hook()
    res = run_bass_kernel_spmd(nc, in_maps, list(range(8)), trace=trace)
    LAST_RESULTS = res

    out = np.zeros((B, T, C), np.float32)
    k_full = np.empty((B, H, T, HD), np.float32)
    v_full = np.empty((B, H, T, HD), np.float32)
    for core in range(8):
        b, g = core // 2, core % 2
        r = res.results[core]
        out[b] += r["outp"]
        k_full[b, g * HG:(g + 1) * HG] = (
            r["kT_out"].reshape(HG, HD, T).transpose(0, 2, 1))
        v_full[b, g * HG:(g + 1) * HG] = (
            r["v_out"].reshape(T, HG, HD).transpose(1, 0, 2))
    return out, k_full, v_full


# revision 15
# speedup vs baseline: 1.2095x; 1.0176x over previous
"""Causal self-attention (B=4, T=2048, C=2048, H=16, rope) on 8 TRN2 NeuronCores.

Sharding: core = (batch b, head-group g) with b in 0..3, g in 0..1. Each core
owns 8 heads of one batch: computes its qkv projection shard, rope, causal
attention, and a partial out-projection (contracting only its 1024 columns of
the 2048-dim head-concat axis). Host sums the two partials per batch
(the "all-reduce after out_proj"), and reassembles k (post-rope) and v.

Per-core dataflow (all matmuls in fp32r on the PE):
  xT [C,T] resident in SBUF.
  v [T,1024] produced in natural layout (lhsT = xT block), written to the v
    output and re-read per head during attention.
  qT,kT [1024,T] produced head-transposed (lhsT = wqkvT panel), rope applied
    at psum-evict time (partition-half swap via SBUF-SBUF DMA), spilled to
    DRAM (the kT spill doubles as the k output).
  Attention per head: S = qT_blk.T @ kT chunks -> causal mask on diag block ->
    softmax (max on DVE, exp+row-sum fused on ACT, 1/sum applied to P) ->
    PE-transpose P blocks -> oT accum = v_blk.T @ P_T, spilled per head.
  out_proj: out[t,o] accumulated over 8 heads (lhsT = oT block, rhs = w_outT).
"""

import os
import sys
import types
import contextlib

sys.path.insert(0, "/opt/trn_rl_repo")

import numpy as np
import ml_dtypes

import concourse.bacc as bacc
import concourse.mybir as mybir
import concourse.tile as tile
from concourse.bass_utils import run_bass_kernel_spmd

F32 = mybir.dt.float32
F32R = mybir.dt.float32r
BF16 = mybir.dt.bfloat16

B, T, C = 4, 2048, 2048
H, HD = 16, 128
HG = 8                      # heads per core
CB = C // 128               # 16 contraction blocks
TB = T // 128               # 16 time blocks
TS = T // 512               # 4 time supertiles
SCALE = 1.0 / np.sqrt(HD)
THETA = 10000.0

LAST_RESULTS = None


def _install_ntff_hook():
    """Make run_bass_kernel_spmd(trace=True) work: register the axon NTFF
    profile hook that the image's antenv package lacks."""
    if "antenv.axon_hooks" in sys.modules:
        return True
    try:
        sys.path.insert(0, "/root/.axon_site")
        from trn_agent_boot.trn_boot import _ntff_profile_via_ctypes
        hook = _ntff_profile_via_ctypes("/opt/axon/libaxon_pjrt.so")
    except Exception:
        return False
    if hook is None:
        return False
    mod = types.ModuleType("antenv.axon_hooks")
    mod._hook = hook
    mod.get_axon_ntff_profile_hook = lambda: mod._hook
    mod.set_axon_ntff_profile_hook = lambda h: setattr(mod, "_hook", h)
    sys.modules["antenv.axon_hooks"] = mod
    import antenv
    antenv.axon_hooks = mod
    return True


def _r(ap):
    return ap.bitcast(F32R)


def build_nc():
    nc = bacc.Bacc(None, target_bir_lowering=False)

    xT_d = nc.declare_dram_parameter("xT", [C, T], BF16, isOutput=False)
    wqkvT_d = nc.declare_dram_parameter("wqkvT", [C, 3 * HG * HD], BF16, isOutput=False)
    w_outT_d = nc.declare_dram_parameter("w_outT", [HG * HD, C], BF16, isOutput=False)
    cosT_d = nc.declare_dram_parameter("cosT", [HD, T], F32, isOutput=False)
    sinT_d = nc.declare_dram_parameter("sinT", [HD, T], F32, isOutput=False)
    mask_d = nc.declare_dram_parameter("mask", [128, 128], F32, isOutput=False)

    outp_d = nc.declare_dram_parameter("outp", [T, C], F32, isOutput=True)
    kT_out_d = nc.declare_dram_parameter("kT_out", [HG * HD, T], F32, isOutput=True)
    v_out_d = nc.declare_dram_parameter("v_out", [T, HG * HD], F32, isOutput=True)

    v16_scr = nc.dram_tensor("v16_scr", [T, HG * HD], BF16)

    with tile.TileContext(nc) as tc:
        qkstk = contextlib.ExitStack()
        qk16pool = qkstk.enter_context(tc.tile_pool(name="qk16pool", bufs=1))
        qk_tiles = [qk16pool.tile([128, T], BF16, tag=f"qk16_{j}",
                                  name=f"qk16_{j}")
                    for j in range(2 * HG)]
        with contextlib.ExitStack() as xstack:
            xpool = xstack.enter_context(tc.tile_pool(name="xpool", bufs=1))
            x_sb = xpool.tile([128, CB, T], BF16, tag="x")
            for ts in range(TS):
                for cb in range(CB):
                    nc.sync.dma_start(
                        out=x_sb[:, cb, ts * 512:(ts + 1) * 512],
                        in_=xT_d[cb * 128:(cb + 1) * 128, ts * 512:(ts + 1) * 512],
                    )

            # ---------------- phase V: v = x @ Wv.T (natural layout) -------
            with contextlib.ExitStack() as ph:
                wvpool = ph.enter_context(tc.tile_pool(name="wvpool", bufs=1))
                psv = ph.enter_context(tc.tile_pool(name="psv", bufs=3, space="PSUM"))
                vev = ph.enter_context(tc.tile_pool(name="vev", bufs=3))
                for vs in range(2):
                    wv_sb = wvpool.tile([128, CB, 512], BF16, tag="wv")
                    for cb in range(CB):
                        nc.sync.dma_start(
                            out=wv_sb[:, cb, :],
                            in_=wqkvT_d[cb * 128:(cb + 1) * 128,
                                        2048 + vs * 512:2048 + (vs + 1) * 512],
                        )
                    for tb in range(TB):
                        ps = psv.tile([128, 512], F32, tag="ps")
                        for cb in range(CB):
                            nc.tensor.matmul(
                                ps[:],
                                (x_sb[:, cb, tb * 128:(tb + 1) * 128]),
                                (wv_sb[:, cb, :]),
                                start=(cb == 0), stop=(cb == CB - 1),
                            )
                        vt = vev.tile([128, 512], F32, tag="vout")
                        nc.scalar.copy(out=vt[:], in_=ps[:])
                        nc.sync.dma_start(
                            out=v_out_d[tb * 128:(tb + 1) * 128,
                                        vs * 512:(vs + 1) * 512],
                            in_=vt[:])
                        vt16 = vev.tile([128, 512], BF16, tag="vout16")
                        nc.vector.tensor_copy(out=vt16[:], in_=ps[:])
                        nc.sync.dma_start(
                            out=v16_scr[tb * 128:(tb + 1) * 128,
                                        vs * 512:(vs + 1) * 512],
                            in_=vt16[:])

            # ---------------- phase QK: qT,kT + rope ----------------------
            with contextlib.ExitStack() as ph:
                trig = ph.enter_context(tc.tile_pool(name="trig", bufs=1))
                cos_sb = trig.tile([HD, T], F32, tag="cos")
                sin_sb = trig.tile([HD, T], F32, tag="sin")
                nc.sync.dma_start(out=cos_sb[:], in_=cosT_d[:])
                nc.sync.dma_start(out=sin_sb[:], in_=sinT_d[:])

                wpool = ph.enter_context(tc.tile_pool(name="wpool", bufs=2))
                psqk = ph.enter_context(
                    tc.tile_pool(name="psqk", bufs=3, space="PSUM"))
                rpool = ph.enter_context(tc.tile_pool(name="rpool", bufs=3))

                # per-head interleave (q0,k0,q1,k1,...) so attention head h
                # only waits for its own two tiles
                for hh in range(HG):
                  for qk in range(2):
                    jb = qk * HG + hh
                    w_sb = wpool.tile([128, CB, 128], BF16, tag="wqk")
                    nc.sync.dma_start(
                        out=w_sb[:],
                        in_=wqkvT_d[:, jb * 128:(jb + 1) * 128].rearrange(
                            "(cb c) j -> c cb j", c=128),
                    )
                    for ts in range(TS):
                        sl = slice(ts * 512, (ts + 1) * 512)
                        ps = psqk.tile([128, 512], F32, tag="ps")
                        for cb in range(CB):
                            nc.tensor.matmul(
                                ps[:], (w_sb[:, cb, :]), (x_sb[:, cb, sl]),
                                start=(cb == 0), stop=(cb == CB - 1),
                            )
                        tmp = rpool.tile([128, 512], F32, tag="tmp")
                        nc.scalar.copy(out=tmp[:], in_=ps[:])
                        rot = rpool.tile([128, 512], F32, tag="rot")
                        nc.sync.dma_start(out=rot[0:64, :], in_=tmp[64:128, :])
                        nc.sync.dma_start(out=rot[64:128, :], in_=tmp[0:64, :])
                        nc.vector.tensor_mul(out=tmp[:], in0=tmp[:], in1=cos_sb[:, sl])
                        nc.vector.tensor_mul(out=rot[:], in0=rot[:], in1=sin_sb[:, sl])
                        nc.vector.tensor_add(out=tmp[:], in0=tmp[:], in1=rot[:])
                        if jb >= HG:
                            nc.sync.dma_start(
                                out=kT_out_d[(jb % HG) * 128:(jb % HG + 1) * 128, sl],
                                in_=tmp[:])
                        nc.vector.tensor_copy(
                            out=qk_tiles[jb][:, sl], in_=tmp[:])

        # ---- attention + out_proj share the oT / w_out SBUF residency ----
        shared = contextlib.ExitStack()
        oipool = shared.enter_context(tc.tile_pool(name="oipool", bufs=1))
        wopool = shared.enter_context(tc.tile_pool(name="wopool", bufs=1))
        oi_sb = oipool.tile([128, HG, T], BF16, tag="oi")
        wo_sb = wopool.tile([128, HG, C], BF16, tag="wo")
        for h in range(HG):
            nc.sync.dma_start(
                out=wo_sb[:, h, :], in_=w_outT_d[h * 128:(h + 1) * 128, :])

        # ---------------- phase A: attention per head ----------------------
        # Transposed-score formulation: S_T[j, i] = k_j.T @ q_i needs no
        # P transposes -- v_sb [j, d] is directly the PV lhsT. Softmax
        # denominators come from an all-ones [128,128] stationary matmul,
        # which also broadcasts them across partitions for free.
        with contextlib.ExitStack() as ph:
            small = ph.enter_context(tc.tile_pool(name="small", bufs=1))
            mask_sb = small.tile([128, 128], F32, tag="mask")
            nc.sync.dma_start(out=mask_sb[:], in_=mask_d[:])
            ones16 = small.tile([128, 128], BF16, tag="ones16")
            nc.vector.memset(ones16[:], 1.0)

            vpool = ph.enter_context(tc.tile_pool(name="vpool", bufs=2))
            stpool = ph.enter_context(tc.tile_pool(name="stpool", bufs=5))
            stats = ph.enter_context(tc.tile_pool(name="stats", bufs=3))
            psS = ph.enter_context(tc.tile_pool(name="psS", bufs=4, space="PSUM"))
            psO = ph.enter_context(tc.tile_pool(name="psO", bufs=2, space="PSUM"))
            psSum = ph.enter_context(tc.tile_pool(name="psSum", bufs=2, space="PSUM"))

            for h in range(HG):
                hsl = slice(h * 128, (h + 1) * 128)
                q_sb = qk_tiles[h]
                k_sb = qk_tiles[HG + h]
                v_sb = vpool.tile([128, TB, 128], BF16, tag="v")
                nc.sync.dma_start(
                    out=v_sb[:],
                    in_=v16_scr[:, hsl].rearrange("(tb t) d -> t tb d", t=128))

                for ig in range(4):
                    igsl = slice(ig * 512, (ig + 1) * 512)
                    njb = 4 * ig + 4
                    po = psO.tile([128, 512], F32, tag="psO")
                    sums = psSum.tile([128, 512], F32, tag="sums")

                    def s_exp(jb):
                        # S_T block -> causal mask on diag -> exp -> bf16 P_T.
                        # In-group blocks only compute the live columns; the
                        # fully-masked prefix of P_T is zeroed instead.
                        p = jb - 4 * ig
                        w0 = max(p, 0) * 128          # first live column
                        ps = psS.tile([128, 512], F32, tag="psS")
                        nc.tensor.matmul(
                            ps[:, w0:],
                            (k_sb[:, jb * 128:(jb + 1) * 128]),
                            (q_sb[:, ig * 512 + w0:(ig + 1) * 512]),
                            start=True, stop=True,
                        )
                        if p >= 0:
                            nc.vector.tensor_add(
                                out=ps[:, w0:w0 + 128],
                                in0=ps[:, w0:w0 + 128], in1=mask_sb[:])
                        s16t = stpool.tile([128, 512], BF16, tag="s16t")
                        if w0 > 0:
                            nc.vector.memset(s16t[:, :w0], 0.0)
                        nc.scalar.activation(
                            out=s16t[:, w0:], in_=ps[:, w0:],
                            func=mybir.ActivationFunctionType.Exp,
                            bias=0.0, scale=SCALE)
                        return s16t

                    def sum_pv(jb, s16t):
                        nc.tensor.matmul(
                            sums[:], (ones16[:]), (s16t[:]),
                            start=(jb == 0), stop=(jb == njb - 1),
                        )
                        nc.tensor.matmul(
                            po[:], (v_sb[:, jb, :]), (s16t[:]),
                            start=(jb == 0), stop=(jb == njb - 1),
                        )

                    # 2-deep software pipeline: S/exp of jb+1 and jb+2 sit
                    # ahead of sum/PV of jb in the PE queue, covering the
                    # exp latency.
                    win = [s_exp(0)]
                    if njb > 1:
                        win.append(s_exp(1))
                    for jb in range(2, njb):
                        win.append(s_exp(jb))
                        sum_pv(jb - 2, win.pop(0))
                    if njb > 1:
                        sum_pv(njb - 2, win.pop(0))
                    sum_pv(njb - 1, win.pop(0))

                    rs = stats.tile([128, 512], F32, tag="rs")
                    nc.vector.reciprocal_approx_fast(out=rs[:], in_=sums[:])
                    nc.vector.tensor_mul(
                        out=oi_sb[:, h, igsl], in0=po[:], in1=rs[:])

        # ---------------- phase O: out projection --------------------------
        with contextlib.ExitStack() as ph:
            psP = ph.enter_context(tc.tile_pool(name="psP", bufs=3, space="PSUM"))
            fpool = ph.enter_context(tc.tile_pool(name="fpool", bufs=3))

            for tb in range(TB):
                for os_ in range(TS):
                    osl = slice(os_ * 512, (os_ + 1) * 512)
                    ps = psP.tile([128, 512], F32, tag="psP")
                    for h in range(HG):
                        nc.tensor.matmul(
                            ps[:],
                            (oi_sb[:, h, tb * 128:(tb + 1) * 128]),
                            (wo_sb[:, h, osl]),
                            start=(h == 0), stop=(h == HG - 1),
                        )
                    ft = fpool.tile([128, 512], F32, tag="fout")
                    nc.scalar.copy(out=ft[:], in_=ps[:])
                    nc.sync.dma_start(
                        out=outp_d[tb * 128:(tb + 1) * 128, osl], in_=ft[:])
        shared.close()
        qkstk.close()

    nc.finalize()
    return nc


_NC_CACHE = None


def _host_tables():
    inv_freq = 1.0 / (THETA ** (np.arange(0, HD, 2, dtype=np.float64) / HD))
    t_ar = np.arange(T, dtype=np.float64)
    emb = np.concatenate([np.outer(t_ar, inv_freq)] * 2, axis=-1)   # [T, 128]
    cosT = np.cos(emb).T.astype(np.float32).copy()
    sinT = np.sin(emb).T.astype(np.float32).copy()
    sinT[:64] *= -1.0
    # transposed causal mask for an S_T diagonal block: mask[j, i] kills
    # keys j > query i
    jj = np.arange(128)[:, None]
    ii = np.arange(128)[None, :]
    mask = np.where(jj > ii, np.float32(-1e30), np.float32(0.0))
    return cosT, sinT, mask


def kernel(x, wqkv, w_out):
    global _NC_CACHE, LAST_RESULTS
    x = np.ascontiguousarray(np.asarray(x, dtype=np.float32))
    wqkv = np.asarray(wqkv, dtype=np.float32)
    w_out = np.asarray(w_out, dtype=np.float32)

    if _NC_CACHE is None:
        _NC_CACHE = build_nc()
    nc = _NC_CACHE

    cosT, sinT, mask = _host_tables()
    in_maps = []
    for core in range(8):
        b, g = core // 2, core % 2
        rows = slice(g * HG * HD, (g + 1) * HG * HD)
        wq = wqkv[0 * C:1 * C][rows]
        wk = wqkv[1 * C:2 * C][rows]
        wv = wqkv[2 * C:3 * C][rows]
        in_maps.append({
            "xT": np.ascontiguousarray(x[b].T).astype(ml_dtypes.bfloat16),
            "wqkvT": np.ascontiguousarray(
                np.concatenate([wq, wk, wv], axis=0).T).astype(ml_dtypes.bfloat16),
            "w_outT": np.ascontiguousarray(
                w_out[:, rows].T).astype(ml_dtypes.bfloat16),
            "cosT": cosT,
            "sinT": sinT,
            "mask": mask,
        })

    trace = bool(os.environ.get("KERNEL_TRACE"))
    if trace:
        _install_ntff_hook()
    res = run_bass_kernel_spmd(nc, in_maps, list(range(8)), trace=trace)
    LAST_RESULTS = res

    out = np.zeros((B, T, C), np.float32)
    k_full = np.empty((B, H, T, HD), np.float32)
    v_full = np.empty((B, H, T, HD), np.float32)
    for core in range(8):
        b, g = core // 2, core % 2
        r = res.results[core]
        out[b] += r["outp"]
        k_full[b, g * HG:(g + 1) * HG] = (
            r["kT_out"].reshape(HG, HD, T).transpose(0, 2, 1))
        v_full[b, g * HG:(g + 1) * HG] = (
            r["v_out"].reshape(T, HG, HD).transpose(1, 0, 2))
    return out, k_full, v_full
